# revision 1
# baseline (speedup 1.0000x reference)
"""Trainium2 Bass kernel for nn_Attention_86268713108190.

7 independent attention "bands" over batch 8, n=512, d=512, 8 heads,
shared Wqkv/Wout. Sharding: data-parallel over batch — core c handles
batch index c (7 band-samples of [512, 512] each).

Per-core dataflow (per sample; all matmuls in float32r: HW-measured
~1.5e-4 matmul rel err at ~387 ns per N=512 matmul vs 853 ns for fp32):
  1. qkvT = Wqkv @ x^T    (lhsT = WqkvT chunks, rhs = x^T)      [e, n]
  2. v    = x @ Wv^T      (lhsT = x^T chunks,   rhs = WvT)      [n, ev]
     v_aug: per head 64 v-cols + a ones column (65) -> softmax
     denominator falls out of the AV matmul for free
  3. per head pair: S^T = k_h q_h^T (K=64), expS^T = exp(SCALE*S^T) on
     ACT (PSUM->SBUF, rounds to f32r; no max-subtraction needed --
     |SCALE*S| <~ 1.1 for this distribution), then
     O_aug^T[65, n] = v_aug.T @ expS^T accumulated over j-tiles;
     row 64 = softmax denominator. Softmax reduction runs over the
     PSUM partition axis via the ones column, so no transposes at all.
  4. normalize tail (fully lagged one head pair, emitted after the
     next pair's S+exp so nothing blocks the in-order PE/ACT streams):
     1/d = exp(-ln d) on ACT (ln+exp share one table set; DVE's
     iterative reciprocal is slow, and reciprocal_approx_fast returns
     garbage on HW despite passing CoreSim), bounced through a DRAM
     scratch tile and broadcast to 64 partitions by a stride-0
     DRAM-source DMA (SBUF stride-0 APs are illegal but DRAM-source
     ones lower fine); two DVE multiplies -> OT [d, n].
  5. out = O @ Wout^T + bias  (lhsT = OT chunks, rhs = WoutT).

Whole-output HW accuracy vs fp32 reference: rel err ~2.9e-4.
Steady-state HW time per core (7 bands): ~500 us in the final A/B
session (measured by For_i repeat differencing; session-to-session
terminal variance is ~+-5%). no_tail ablation floor is ~364 us; the
residual gap is the normalize tail's DVE reciprocal + cross-engine
chain, which measured equal across GpSimd/PE-matmul/DMA broadcast
mechanisms, lagged or not. HW ablations: QKV+out-proj alone run at
~133 us, attention S/exp/AV adds ~170 us, and the softmax-normalize
tail adds the rest -- its cross-engine latency chain is the main
non-PE cost; PE-stream (mask-matmul) and lagged variants measured
slower than the off-stream GpSimd broadcast.
"""

import contextlib
import sys

if '/opt/trn_rl_repo' not in sys.path:
    sys.path.insert(0, '/opt/trn_rl_repo')

import numpy as np

P = 128
MM_DTYPE = "f32r"
NSEQ = 512
D = 512
H = 8
DH = 64
NBANDS = 7
NCORES = 8
SCALE = D ** -0.5

_cached = None


def _emit_band(ctx, s, xt):
    """Emit one band's compute. `xt` is the (already DMA'd) x^T tile."""
    nc, f32, f32r, Exp = ctx["nc"], ctx["f32"], ctx["f32r"], ctx["Exp"]
    wq_sb, wo_sb, bias_sb = ctx["wq_sb"], ctx["wo_sb"], ctx["bias_sb"]
    out = ctx["out"]
    pl = ctx["pools"]

    # --- QKV projections -> qkvT layout for q,k ---
    qk_sb = pl["qk"].tile([P, 8, NSEQ], f32r, tag="qk")
    for et in (0, 4, 1, 5, 2, 6, 3, 7):
        ps = pl["psproj"].tile([P, NSEQ], f32, tag="psproj")
        for kt in range(4):
            nc.tensor.matmul(
                ps[:], wq_sb[:, kt, et * P:(et + 1) * P], xt[:, kt, :],
                start=(kt == 0), stop=(kt == 3))
        nc.vector.tensor_copy(qk_sb[:, et, :], ps[:])

    # --- V projection -> row-major v_aug with ones column ---
    v_aug = pl["v"].tile([P, 4, H, DH + 1], f32r, tag="vaug")
    for nt in range(4):
        ps = pl["psproj"].tile([P, NSEQ], f32, tag="psproj")
        for kt in range(4):
            nc.tensor.matmul(
                ps[:], xt[:, kt, nt * P:(nt + 1) * P],
                wq_sb[:, kt, 2 * D:3 * D],
                start=(kt == 0), stop=(kt == 3))
        nc.vector.tensor_copy(
            v_aug[:, nt, :, 0:DH],
            ps[:].rearrange("p (h dh) -> p h dh", h=H))
        ones_slice = v_aug[:, nt, :, DH:DH + 1]
        if ctx["mm_dtype"] == "f32r":
            ones_slice = ones_slice.bitcast(f32)
        nc.vector.memset(ones_slice, 1.0)

    # --- attention per head pair (2g, 2g+1) ---
    # Three emission orders were measured on HW; "split" (all S+exp of a
    # pair, then its AVs) was fastest by a small margin.
    ot_sb = pl["ot"].tile([P, 4, NSEQ], f32r, tag="ot")
    es_store = {}

    def s_phase(g):
        es_list = []
        for jt in range(4):
            ps_s0 = pl["pss"].tile([P, NSEQ], f32, tag="pss")
            ps_s1 = pl["pss"].tile([P, NSEQ], f32, tag="pss")
            nc.tensor.matmul(
                ps_s0[:],
                qk_sb[0:DH, 4 + g, jt * P:(jt + 1) * P],
                qk_sb[0:DH, g, :], start=True, stop=True)
            nc.tensor.matmul(
                ps_s1[:],
                qk_sb[DH:P, 4 + g, jt * P:(jt + 1) * P],
                qk_sb[DH:P, g, :], start=True, stop=True,
                tile_position=(DH, 0))
            es = pl["es"].tile([P, 2, NSEQ], f32r, tag="es")
            nc.scalar.activation(es[:, 0, :], ps_s0[:], Exp, scale=SCALE)
            nc.scalar.activation(es[:, 1, :], ps_s1[:], Exp, scale=SCALE)
            es_list.append(es)
        es_store[g] = es_list

    tail_store = {}

    def av_phase(g):
        es_list = es_store.pop(g)
        if ctx["ablate"] == "no_av":
            nc.vector.tensor_copy(ot_sb[:, g, :], es_list[0][:, 0, :])
            return
        ps_o0 = pl["pso"].tile([DH + 1, NSEQ], f32, tag="pso")
        ps_o1 = pl["pso"].tile([DH + 1, NSEQ], f32, tag="pso")
        for jt in range(4):
            nc.tensor.matmul(
                ps_o0[:], v_aug[:, jt, 2 * g, :], es_list[jt][:, 0, :],
                start=(jt == 0), stop=(jt == 3))
            nc.tensor.matmul(
                ps_o1[:], v_aug[:, jt, 2 * g + 1, :], es_list[jt][:, 1, :],
                start=(jt == 0), stop=(jt == 3))
        if ctx["ablate"] == "no_tail":
            nc.vector.tensor_copy(ot_sb[0:DH, g, :], ps_o0[0:DH, :])
            nc.vector.tensor_copy(ot_sb[DH:P, g, :], ps_o1[0:DH, :])
            return
        if ctx["tail"] == "pbcast":
            rcc = pl["r"].tile([1, 2 * NSEQ], f32, tag="rcc")
            nc.vector.reciprocal(rcc[0:1, 0:NSEQ], ps_o0[DH:DH + 1, :])
            nc.vector.reciprocal(rcc[0:1, NSEQ:2 * NSEQ],
                                 ps_o1[DH:DH + 1, :])
            rb = pl["r"].tile([DH, 2 * NSEQ], f32, tag="rb")
            nc.gpsimd.partition_broadcast(rb[:], rcc[:])
            nc.vector.tensor_mul(ot_sb[0:DH, g, :], ps_o0[0:DH, :],
                                 rb[:, 0:NSEQ])
            nc.vector.tensor_mul(ot_sb[DH:P, g, :], ps_o1[0:DH, :],
                                 rb[:, NSEQ:2 * NSEQ])
            return
        if ctx["tail"] == "dma":
            # whole tail is computed lagged in tail_finish, after the
            # NEXT pair's S+exp, so the ACT Ln/Exp reciprocal never
            # blocks the softmax exps in ACT's in-order stream.
            tail_store[g] = (ps_o0, ps_o1, None, "lagall")
            return
        # lagged PE mask-matmul tail: recips now, broadcast+mults under
        # the next pair's S matmuls.
        rc0 = pl["r"].tile([1, NSEQ], f32, tag="rc0")
        rc1 = pl["r"].tile([1, NSEQ], f32, tag="rc1")
        nc.vector.reciprocal(rc0[:], ps_o0[DH:DH + 1, :])
        nc.vector.reciprocal(rc1[:], ps_o1[DH:DH + 1, :])
        tail_store[g] = (ps_o0, ps_o1, rc0, rc1)

    def tail_finish(g):
        if g not in tail_store:
            return
        ps_o0, ps_o1, rc0, rc1 = tail_store.pop(g)
        if rc1 == "lagall":
            # 1/d = exp(-ln d) on ACT (ln+exp share the
            # natural_log_exp_and_others table set); DRAM-bounce DMA
            # broadcast; DVE multiplies.
            lg = pl["r"].tile([1, 2 * NSEQ], f32, tag="lg")
            Ln = ctx["Ln"]
            nc.scalar.activation(lg[0:1, 0:NSEQ], ps_o0[DH:DH + 1, :], Ln)
            nc.scalar.activation(lg[0:1, NSEQ:2 * NSEQ],
                                 ps_o1[DH:DH + 1, :], Ln)
            rcc = pl["r"].tile([1, 2 * NSEQ], f32, tag="rcc")
            nc.scalar.activation(rcc[:], lg[:], Exp, scale=-1.0)
            dr = pl["dram"].tile([1, 2 * NSEQ], f32, tag="dr")
            nc.sync.dma_start(dr[:], rcc[:])
            rb = pl["r"].tile([DH, 2 * NSEQ], f32, tag="rb")
            nc.sync.dma_start(rb[:], dr[:].to_broadcast((DH, 2 * NSEQ)))
            nc.vector.tensor_mul(ot_sb[0:DH, g, :], ps_o0[0:DH, :],
                                 rb[:, 0:NSEQ])
            nc.vector.tensor_mul(ot_sb[DH:P, g, :], ps_o1[0:DH, :],
                                 rb[:, NSEQ:2 * NSEQ])
            return
        if rc1 == "dma":  # rc0 is a DRAM [1, 2*NSEQ] recip row
            rb = pl["r"].tile([DH, 2 * NSEQ], f32, tag="rb")
            nc.sync.dma_start(rb[:], rc0[:].to_broadcast((DH, 2 * NSEQ)))
            nc.vector.tensor_mul(ot_sb[0:DH, g, :], ps_o0[0:DH, :],
                                 rb[:, 0:NSEQ])
            nc.vector.tensor_mul(ot_sb[DH:P, g, :], ps_o1[0:DH, :],
                                 rb[:, NSEQ:2 * NSEQ])
            return
        if rc1 is None:  # pblag: rc0 is the merged [1, 1024] recip row
            rb = pl["r"].tile([DH, 2 * NSEQ], f32, tag="rb")
            nc.gpsimd.partition_broadcast(rb[:], rc0[:])
            nc.vector.tensor_mul(ot_sb[0:DH, g, :], ps_o0[0:DH, :],
                                 rb[:, 0:NSEQ])
            nc.vector.tensor_mul(ot_sb[DH:P, g, :], ps_o1[0:DH, :],
                                 rb[:, NSEQ:2 * NSEQ])
            return
        rb_ps = pl["psproj"].tile([P, NSEQ], f32, tag="psproj")
        nc.tensor.matmul(rb_ps[:], ctx["maskA"][:], rc0[:],
                         start=True, stop=False)
        nc.tensor.matmul(rb_ps[:], ctx["maskB"][:], rc1[:],
                         start=False, stop=True)
        rb_sb = pl["r"].tile([P, NSEQ], f32, tag="rbsb")
        nc.vector.tensor_copy(rb_sb[:], rb_ps[:])
        nc.vector.tensor_mul(ot_sb[0:DH, g, :], ps_o0[0:DH, :],
                             rb_sb[0:DH, :])
        nc.vector.tensor_mul(ot_sb[DH:P, g, :], ps_o1[0:DH, :],
                             rb_sb[DH:P, :])

    def interleaved_pair(g):
        ps_o0 = pl["pso"].tile([DH + 1, NSEQ], f32, tag="pso")
        ps_o1 = pl["pso"].tile([DH + 1, NSEQ], f32, tag="pso")
        for jt in range(4):
            ps_s0 = pl["pss"].tile([P, NSEQ], f32, tag="pss")
            ps_s1 = pl["pss"].tile([P, NSEQ], f32, tag="pss")
            nc.tensor.matmul(
                ps_s0[:],
                qk_sb[0:DH, 4 + g, jt * P:(jt + 1) * P],
                qk_sb[0:DH, g, :], start=True, stop=True)
            nc.tensor.matmul(
                ps_s1[:],
                qk_sb[DH:P, 4 + g, jt * P:(jt + 1) * P],
                qk_sb[DH:P, g, :], start=True, stop=True,
                tile_position=(DH, 0))
            es = pl["es"].tile([P, 2, NSEQ], f32r, tag="es")
            nc.scalar.activation(es[:, 0, :], ps_s0[:], Exp, scale=SCALE)
            nc.scalar.activation(es[:, 1, :], ps_s1[:], Exp, scale=SCALE)
            nc.tensor.matmul(
                ps_o0[:], v_aug[:, jt, 2 * g, :], es[:, 0, :],
                start=(jt == 0), stop=(jt == 3))
            nc.tensor.matmul(
                ps_o1[:], v_aug[:, jt, 2 * g + 1, :], es[:, 1, :],
                start=(jt == 0), stop=(jt == 3))
        rc0 = pl["r"].tile([1, NSEQ], f32, tag="rc0")
        rc1 = pl["r"].tile([1, NSEQ], f32, tag="rc1")
        nc.vector.reciprocal(rc0[:], ps_o0[DH:DH + 1, :])
        nc.vector.reciprocal(rc1[:], ps_o1[DH:DH + 1, :])
        rb0 = pl["r"].tile([DH, NSEQ], f32, tag="rb0")
        rb1 = pl["r"].tile([DH, NSEQ], f32, tag="rb1")
        nc.gpsimd.partition_broadcast(rb0[:], rc0[:])
        nc.gpsimd.partition_broadcast(rb1[:], rc1[:])
        nc.vector.tensor_mul(ot_sb[0:DH, g, :], ps_o0[0:DH, :], rb0[:])
        nc.vector.tensor_mul(ot_sb[DH:P, g, :], ps_o1[0:DH, :], rb1[:])

    if ctx["ablate"] == "no_attn":
        nc.vector.tensor_copy(ot_sb[:], qk_sb[:, 0:4, :])
    elif ctx["ablate"] == "no_exp":
        for g in range(4):
            for jt in range(4):
                ps_s0 = pl["pss"].tile([P, NSEQ], f32, tag="pss")
                ps_s1 = pl["pss"].tile([P, NSEQ], f32, tag="pss")
                nc.tensor.matmul(
                    ps_s0[:], qk_sb[0:DH, 4 + g, jt * P:(jt + 1) * P],
                    qk_sb[0:DH, g, :], start=True, stop=True)
                nc.tensor.matmul(
                    ps_s1[:], qk_sb[DH:P, 4 + g, jt * P:(jt + 1) * P],
                    qk_sb[DH:P, g, :], start=True, stop=True,
                    tile_position=(DH, 0))
                es = pl["es"].tile([P, 2, NSEQ], f32r, tag="es")
                nc.vector.tensor_copy(es[:, 0, :], ps_s0[:])
                nc.vector.tensor_copy(es[:, 1, :], ps_s1[:])
                es_store.setdefault(g, []).append(es)
            av_phase(g)
    elif ctx["pipe"] == "pipe":
        s_phase(0)
        for g in range(1, 4):
            s_phase(g)
            av_phase(g - 1)
        av_phase(3)
    elif ctx["pipe"] == "split":
        for g in range(4):
            s_phase(g)
            tail_finish(g - 1)
            av_phase(g)
        tail_finish(3)
    else:  # "v2": exp and AV interleaved per j-tile
        for g in range(4):
            interleaved_pair(g)

    # --- output projection + bias ---
    for nt in range(4):
        ps = pl["psproj"].tile([P, NSEQ], f32, tag="psproj")
        for kt in range(4):
            nc.tensor.matmul(
                ps[:], ot_sb[:, kt, nt * P:(nt + 1) * P], wo_sb[:, kt, :],
                start=(kt == 0), stop=(kt == 3))
        ob = pl["ob"].tile([P, D], f32, tag="ob")
        nc.vector.tensor_add(ob[:], ps[:], bias_sb[:])
        nc.sync.dma_start(
            out[s].rearrange("(no ni) e -> ni no e", ni=P)[:, nt, :], ob[:])


def build_kernel(nbands=NBANDS, repeat=1, mm_dtype=MM_DTYPE, pipe="split", ablate="", tail="dma"):
    import concourse.mybir as mybir
    import concourse.tile as tile
    from concourse import bacc
    from concourse import library_config

    f32 = mybir.dt.float32
    f32r = (mybir.dt.float32r if mm_dtype == "f32r" else mybir.dt.bfloat16)
    Exp = mybir.ActivationFunctionType.Exp
    Ln = mybir.ActivationFunctionType.Ln

    nc = bacc.Bacc("TRN2", target_bir_lowering=False, debug=False,
                   num_devices=NCORES)

    xT = nc.dram_tensor("xT", [nbands, D, NSEQ], f32r, kind="ExternalInput").ap()
    wqkvT = nc.dram_tensor("wqkvT", [D, 3 * D], f32r, kind="ExternalInput").ap()
    woutT = nc.dram_tensor("woutT", [D, D], f32r, kind="ExternalInput").ap()
    biasb = nc.dram_tensor("biasb", [P, D], f32, kind="ExternalInput").ap()
    out = nc.dram_tensor("out", [nbands, NSEQ, D], f32, kind="ExternalOutput").ap()

    nc.gpsimd.load_library(library_config.attn)

    with tile.TileContext(nc) as tc:
        with (
            tc.tile_pool(name="weights", bufs=1) as wpool,
            tc.tile_pool(name="x", bufs=3) as xpool,
            tc.tile_pool(name="qk", bufs=2) as qkpool,
            tc.tile_pool(name="v", bufs=2) as vpool,
            tc.tile_pool(name="ot", bufs=2) as otpool,
            tc.tile_pool(name="es", bufs=8) as spool,
            tc.tile_pool(name="r", bufs=3) as rpool,
            tc.tile_pool(name="ob", bufs=3) as outpool,
            tc.tile_pool(name="dram", bufs=3, space="DRAM") as drampool,
            tc.tile_pool(name="psproj", bufs=2, space="PSUM") as psproj,
            tc.tile_pool(name="pss", bufs=2, space="PSUM") as pss,
            tc.tile_pool(name="pso", bufs=4, space="PSUM") as pso,
        ):
            # weights: split wq by k-chunk so the first matmuls can start
            # as soon as their chunk lands
            wq_sb = wpool.tile([P, 4, 3 * D], f32r)
            wo_sb = wpool.tile([P, 4, D], f32r)
            bias_sb = wpool.tile([P, D], f32)
            maskA = wpool.tile([1, P], f32)
            maskB = wpool.tile([1, P], f32)
            nc.vector.memset(maskA[:], 0.0)
            nc.vector.memset(maskB[:], 0.0)
            nc.vector.memset(maskA[0:1, 0:DH], 1.0)
            nc.vector.memset(maskB[0:1, DH:P], 1.0)
            wq_r = wqkvT.rearrange("(ko ki) e -> ki ko e", ki=P)
            for kt in range(4):
                nc.sync.dma_start(wq_sb[:, kt, :], wq_r[:, kt, :])
            nc.sync.dma_start(wo_sb[:], woutT.rearrange("(ko ki) e -> ki ko e", ki=P))
            nc.sync.dma_start(bias_sb[:], biasb[:])

            ctx = {
                "nc": nc, "f32": f32, "f32r": f32r, "Exp": Exp, "Ln": Ln,
                "mm_dtype": mm_dtype, "pipe": pipe, "ablate": ablate, "tail": tail,
                "maskA": maskA, "maskB": maskB,
                "wq_sb": wq_sb, "wo_sb": wo_sb, "bias_sb": bias_sb,
                "out": out,
                "pools": {
                    "qk": qkpool, "v": vpool, "ot": otpool, "es": spool,
                    "r": rpool, "ob": outpool, "psproj": psproj,
                    "dram": drampool,
                    "pss": pss, "pso": pso,
                },
            }

            def load_x(s):
                xt = xpool.tile([P, 4, NSEQ], f32r, tag="xt")
                nc.sync.dma_start(
                    xt[:], xT[s].rearrange("(ko ki) n -> ki ko n", ki=P))
                return xt

            rep_ctx = (tc.For_i(0, repeat, 1,
                                hint_engines=(mybir.EngineType.PE,
                                              mybir.EngineType.Activation,
                                              mybir.EngineType.DVE))
                       if repeat > 1 else contextlib.nullcontext())
            with rep_ctx:
                # prefetch x one band ahead
                xt_next = load_x(0)
                for s in range(nbands):
                    xt = xt_next
                    if s + 1 < nbands:
                        xt_next = load_x(s + 1)
                    _emit_band(ctx, s, xt)

    nc.compile()
    return nc


def _get_nc():
    global _cached
    if _cached is None:
        _cached = build_kernel()
    return _cached


def make_in_maps(x, x_delta, x_theta, x_alpha, x_beta, x_gamma, x_upper,
                 Wqkv, Wout, bout, mm_dtype=MM_DTYPE):
    if mm_dtype == "f32r":
        cast_dt = np.float32
    else:
        import ml_dtypes
        cast_dt = ml_dtypes.bfloat16
    xs = np.stack([np.asarray(a, dtype=np.float32) for a in
                   (x, x_delta, x_theta, x_alpha, x_beta, x_gamma, x_upper)],
                  axis=0)  # [7, b, n, d]
    xsT = np.ascontiguousarray(xs.transpose(1, 0, 3, 2).astype(cast_dt))
    wqkvT = np.ascontiguousarray(np.asarray(Wqkv, np.float32).T.astype(cast_dt))
    woutT = np.ascontiguousarray(np.asarray(Wout, np.float32).T.astype(cast_dt))
    biasb = np.ascontiguousarray(
        np.broadcast_to(np.asarray(bout, np.float32)[None, :], (P, D)))
    return [
        {"xT": xsT[c], "wqkvT": wqkvT, "woutT": woutT, "biasb": biasb}
        for c in range(NCORES)
    ]


def kernel(x, x_delta, x_theta, x_alpha, x_beta, x_gamma, x_upper,
           Wqkv, Wout, bout):
    from concourse.bass_utils import run_bass_kernel_spmd

    nc = _get_nc()
    in_maps = make_in_maps(x, x_delta, x_theta, x_alpha, x_beta, x_gamma,
                           x_upper, Wqkv, Wout, bout)
    res = run_bass_kernel_spmd(nc, in_maps, core_ids=list(range(NCORES)))
    full = np.empty((NBANDS, NCORES, NSEQ, D), dtype=np.float32)
    for c in range(NCORES):
        full[:, c] = res.results[c]["out"]
    return tuple(full[i] for i in range(NBANDS))



# revision 4
# speedup vs baseline: 1.4848x; 1.4848x over previous
"""Trainium2 Bass kernel for nn_Attention_86268713108190.

7 independent attention "bands" over batch 8, n=512, d=512, 8 heads,
shared Wqkv/Wout. Sharding: data-parallel over batch — core c handles
batch index c (7 band-samples of [512, 512] each).

v2 design (vs the 551us f32r baseline):
  * all matmuls in bf16 (PE streams 1 col/cycle @2.4GHz vs ~2 for f32r;
    rel-err budget is 2e-2, bf16 lands ~1e-3).
  * softmax denominator via 64 REPLICATED ones-columns in v_aug: the AV
    matmul out is [128, 512] with rows 0:64 = O_h^T and rows 64:128 =
    den_h broadcast across 64 partitions. Normalization is then a single
    DVE tensor_tensor divide per half-pair (PSUM rows 0:64 / rows
    64:128 -> SBUF), with zero partition-broadcasts, no DRAM bounce, no
    gpsimd, and zero extra PE streaming time (matmul cost is N-driven).
  * software-pipelined emission: PE's in-order stream interleaves, per
    attention jt-slot of band s, the S-pair matmul (g), the AV-pair
    matmul (g-1), and one filler chunk — a QKV projection group of band
    s+1 or an out-projection group of band s-1 — so the PE stays busy
    while ACT (the attention-phase pacer at ~1.2us/jt of exps) keeps up.

Per-band engine budgets (bf16): PE ~24us (112 matmuls, S pairs packed
2x via tile_position row-groups), ACT ~20us (32 exps of [128,512]),
DVE ~16us (evictions + divides). PSUM: psproj 2 + pss 2 + pso 4 = 8.
"""

import contextlib
import sys

if '/opt/trn_rl_repo' not in sys.path:
    sys.path.insert(0, '/opt/trn_rl_repo')

import numpy as np

P = 128
MM_DTYPE = "bf16"
NSEQ = 512
D = 512
H = 8
DH = 64
NBANDS = 7
NCORES = 8
SCALE = D ** -0.5


def build_kernel(nbands=NBANDS, repeat=1, mm_dtype=MM_DTYPE, interleave=True,
                 ablate=""):
    import concourse.mybir as mybir
    import concourse.tile as tile
    from concourse import bacc

    f32 = mybir.dt.float32
    if mm_dtype == "bf16":
        mdt = mybir.dt.bfloat16
    elif mm_dtype == "f32r":
        mdt = mybir.dt.float32r
    else:
        mdt = mybir.dt.float32
    Exp = mybir.ActivationFunctionType.Exp

    nc = bacc.Bacc("TRN2", target_bir_lowering=False, debug=False,
                   num_devices=NCORES)

    xT = nc.dram_tensor("xT", [nbands, D, NSEQ], mdt, kind="ExternalInput").ap()
    wqkvT = nc.dram_tensor("wqkvT", [D, 3 * D], mdt, kind="ExternalInput").ap()
    woutT = nc.dram_tensor("woutT", [D, D], mdt, kind="ExternalInput").ap()
    biasb = nc.dram_tensor("biasb", [P, D], f32, kind="ExternalInput").ap()
    out = nc.dram_tensor("out", [nbands, NSEQ, D], f32, kind="ExternalOutput").ap()

    with tile.TileContext(nc) as tc:
        with (
            tc.tile_pool(name="weights", bufs=1) as wpool,
            tc.tile_pool(name="x", bufs=3) as xpool,
            tc.tile_pool(name="qk", bufs=2) as qkpool,
            tc.tile_pool(name="ot", bufs=2) as otpool,
            tc.tile_pool(name="es", bufs=8) as spool,
            tc.tile_pool(name="ob", bufs=3) as outpool,
            tc.tile_pool(name="psproj", bufs=2, space="PSUM") as psproj,
            tc.tile_pool(name="pss", bufs=2, space="PSUM") as pss,
            tc.tile_pool(name="pso", bufs=2, space="PSUM") as pso,
            tc.tile_pool(name="rec", bufs=2) as recpool,
        ):
            wq_sb = wpool.tile([P, 4, 3 * D], mdt, name="wq_sb")
            wo_sb = wpool.tile([P, 4, D], mdt, name="wo_sb")
            bias_sb = wpool.tile([P, D], f32, name="bias_sb")
            # v_aug: per head 64 V columns + 64 ones columns, so the AV
            # matmul lands O^T on partitions 0:64 and the softmax
            # denominator REPLICATED on partitions 64:128. Two persistent
            # buffers alternated by band parity; ones half memset once.
            vas = [wpool.tile([P, 4, H, 2 * DH], mdt, name=f"va{i}")
                   for i in range(2)]
            for va in vas:
                ones = va[:, :, :, DH:2 * DH]
                if mm_dtype == "f32r":
                    ones = ones.bitcast(f32)
                nc.vector.memset(ones, 1.0)

            wq_r = wqkvT.rearrange("(ko ki) e -> ki ko e", ki=P)
            for kt in range(4):
                nc.sync.dma_start(wq_sb[:, kt, :], wq_r[:, kt, :])
            nc.sync.dma_start(wo_sb[:], woutT.rearrange("(ko ki) e -> ki ko e", ki=P))
            nc.sync.dma_start(bias_sb[:], biasb[:])

            def load_x(s):
                xt = xpool.tile([P, 4, NSEQ], mdt, tag="xt", name="xt")
                nc.sync.dma_start(
                    xt[:], xT[s].rearrange("(ko ki) n -> ki ko n", ki=P))
                return xt

            def qk_group(xt, qk_sb, et):
                ps = psproj.tile([P, NSEQ], f32, tag="psproj", name="ps")
                for kt in range(4):
                    nc.tensor.matmul(
                        ps[:], wq_sb[:, kt, et * P:(et + 1) * P], xt[:, kt, :],
                        start=(kt == 0), stop=(kt == 3))
                nc.vector.tensor_copy(qk_sb[:, et, :], ps[:])

            def v_group(xt, va, nt):
                ps = psproj.tile([P, NSEQ], f32, tag="psproj", name="ps")
                for kt in range(4):
                    nc.tensor.matmul(
                        ps[:], xt[:, kt, nt * P:(nt + 1) * P],
                        wq_sb[:, kt, 2 * D:3 * D],
                        start=(kt == 0), stop=(kt == 3))
                nc.vector.tensor_copy(
                    va[:, nt, :, 0:DH],
                    ps[:].rearrange("p (h dh) -> p h dh", h=H))

            def out_group(s, ot_sb, nt):
                ps = psproj.tile([P, NSEQ], f32, tag="psproj", name="ps")
                for kt in range(4):
                    nc.tensor.matmul(
                        ps[:], ot_sb[:, kt, nt * P:(nt + 1) * P],
                        wo_sb[:, kt, :],
                        start=(kt == 0), stop=(kt == 3))
                ob = outpool.tile([P, D], f32, tag="ob", name="ob")
                nc.vector.tensor_add(ob[:], ps[:], bias_sb[:])
                nc.sync.dma_start(
                    out[s].rearrange("(no ni) e -> ni no e", ni=P)[:, nt, :],
                    ob[:])

            def qkv_chunks(s, xt, va):
                # q0,k0 first so the next band's pair-0 S can start early.
                for et in (0, 4, 1, 5, 2, 6, 3, 7):
                    yield lambda et=et: qk_group(xt, qk_sb_of[s], et)
                for nt in range(4):
                    yield lambda nt=nt: v_group(xt, va, nt)

            qk_sb_of = {}

            def emit_attention(s, qk_sb, va, filler):
                """S/exp for pair g interleaved with AV for pair g-1 and one
                filler chunk per jt slot; divides (the whole softmax
                normalize) inline on DVE."""
                es_store = {}
                ps_store = {}

                def s_pair(g, jt):
                    ps_s0 = pss.tile([P, NSEQ], f32, tag="pss", name="ps_s0")
                    ps_s1 = pss.tile([P, NSEQ], f32, tag="pss", name="ps_s1")
                    nc.tensor.matmul(
                        ps_s0[:], qk_sb[0:DH, 4 + g, jt * P:(jt + 1) * P],
                        qk_sb[0:DH, g, :], start=True, stop=True)
                    nc.tensor.matmul(
                        ps_s1[:], qk_sb[DH:P, 4 + g, jt * P:(jt + 1) * P],
                        qk_sb[DH:P, g, :], start=True, stop=True,
                        tile_position=(DH, 0))
                    es = spool.tile([P, 2, NSEQ], mdt, tag="es", name="es")
                    nc.scalar.activation(es[:, 0, :], ps_s0[:], Exp, scale=SCALE)
                    nc.scalar.activation(es[:, 1, :], ps_s1[:], Exp, scale=SCALE)
                    es_store.setdefault(g, []).append(es)

                def av_pair(g, jt):
                    if jt == 0:
                        ps_store[g] = pso.tile([P, 2, NSEQ], f32, tag="pso",
                                               name="ps_o")
                    ps_o = ps_store[g]
                    es = es_store[g][jt]
                    nc.tensor.matmul(
                        ps_o[:, 0, :], va[:, jt, 2 * g, :], es[:, 0, :],
                        start=(jt == 0), stop=(jt == 3))
                    nc.tensor.matmul(
                        ps_o[:, 1, :], va[:, jt, 2 * g + 1, :], es[:, 1, :],
                        start=(jt == 0), stop=(jt == 3))

                def divides(g):
                    # softmax normalize: rows 64:128 of ps_o hold the
                    # denominator replicated across 64 partitions, so this
                    # is a plain elementwise recip+mul -- no broadcasts.
                    del es_store[g]
                    ps_o = ps_store.pop(g)
                    rec = recpool.tile([DH, 2, NSEQ], f32, tag="rec",
                                       name="rec")
                    nc.vector.reciprocal(rec[:], ps_o[DH:P, :, :])
                    nc.vector.tensor_mul(ot_sb[0:DH, g, :], ps_o[0:DH, 0, :],
                                         rec[:, 0, :])
                    nc.vector.tensor_mul(ot_sb[DH:P, g, :], ps_o[0:DH, 1, :],
                                         rec[:, 1, :])

                ot_sb = otpool.tile([P, 4, NSEQ], mdt, tag="ot", name="ot_sb")
                if ablate == "no_attn":
                    nc.vector.tensor_copy(ot_sb[:], qk_sb[:, 0:4, :])
                    for f in filler:
                        f()
                    return ot_sb

                for g in range(5):
                    for jt in range(4):
                        if g < 4:
                            s_pair(g, jt)
                        if g >= 1:
                            av_pair(g - 1, jt)
                        f = next(filler, None)
                        if f is not None:
                            f()
                    if g >= 1:
                        divides(g - 1)
                for f in filler:
                    f()
                return ot_sb

            rep_ctx = (tc.For_i(0, repeat, 1,
                                hint_engines=(mybir.EngineType.PE,
                                              mybir.EngineType.Activation,
                                              mybir.EngineType.DVE))
                       if repeat > 1 else contextlib.nullcontext())
            with rep_ctx:
                xts = {0: load_x(0)}
                if nbands > 1:
                    xts[1] = load_x(1)
                # prologue: band 0's QKV emitted straight
                qk_sb_of[0] = qkpool.tile([P, 8, NSEQ], mdt, tag="qk",
                                          name="qk_sb")
                for f in qkv_chunks(0, xts[0], vas[0]):
                    f()
                ot_prev = None
                for s in range(nbands):
                    if s + 2 < nbands:
                        xts[s + 2] = load_x(s + 2)
                    filler = []
                    if interleave:
                        if s + 1 < nbands:
                            qk_sb_of[s + 1] = qkpool.tile(
                                [P, 8, NSEQ], mdt, tag="qk", name="qk_sb")
                            filler.append(qkv_chunks(s + 1, xts[s + 1],
                                                     vas[(s + 1) % 2]))
                        if ot_prev is not None:
                            op, os_ = ot_prev
                            filler.append(
                                lambda nt=nt, op=op, os_=os_: out_group(os_, op, nt)
                                for nt in range(4))
                    fill_iter = (f for fl in filler for f in fl)
                    ot = emit_attention(s, qk_sb_of.pop(s), vas[s % 2],
                                        fill_iter)
                    if not interleave:
                        if s + 1 < nbands:
                            qk_sb_of[s + 1] = qkpool.tile(
                                [P, 8, NSEQ], mdt, tag="qk", name="qk_sb")
                            for f in qkv_chunks(s + 1, xts[s + 1],
                                                vas[(s + 1) % 2]):
                                f()
                        if ot_prev is not None:
                            op, os_ = ot_prev
                            for nt in range(4):
                                out_group(os_, op, nt)
                    ot_prev = (ot, s)
                    xts.pop(s, None)
                # epilogue: last band's out-projection
                op, os_ = ot_prev
                for nt in range(4):
                    out_group(os_, op, nt)

    nc.compile()
    return nc


_cached = None


def _get_nc():
    global _cached
    if _cached is None:
        _cached = build_kernel()
    return _cached


def make_in_maps(x, x_delta, x_theta, x_alpha, x_beta, x_gamma, x_upper,
                 Wqkv, Wout, bout, mm_dtype=MM_DTYPE):
    if mm_dtype == "bf16":
        import ml_dtypes
        cast_dt = ml_dtypes.bfloat16
    else:
        cast_dt = np.float32
    xs = np.stack([np.asarray(a, dtype=np.float32) for a in
                   (x, x_delta, x_theta, x_alpha, x_beta, x_gamma, x_upper)],
                  axis=0)  # [7, b, n, d]
    xsT = np.ascontiguousarray(xs.transpose(1, 0, 3, 2).astype(cast_dt))
    wqkvT = np.ascontiguousarray(np.asarray(Wqkv, np.float32).T.astype(cast_dt))
    woutT = np.ascontiguousarray(np.asarray(Wout, np.float32).T.astype(cast_dt))
    biasb = np.ascontiguousarray(
        np.broadcast_to(np.asarray(bout, np.float32)[None, :], (P, D)))
    return [
        {"xT": xsT[c], "wqkvT": wqkvT, "woutT": woutT, "biasb": biasb}
        for c in range(NCORES)
    ]


def kernel(x, x_delta, x_theta, x_alpha, x_beta, x_gamma, x_upper,
           Wqkv, Wout, bout):
    from concourse.bass_utils import run_bass_kernel_spmd

    nc = _get_nc()
    in_maps = make_in_maps(x, x_delta, x_theta, x_alpha, x_beta, x_gamma,
                           x_upper, Wqkv, Wout, bout)
    res = run_bass_kernel_spmd(nc, in_maps, core_ids=list(range(NCORES)))
    full = np.empty((NBANDS, NCORES, NSEQ, D), dtype=np.float32)
    for c in range(NCORES):
        full[:, c] = res.results[c]["out"]
    return tuple(full[i] for i in range(NBANDS))


# revision 12
# speedup vs baseline: 2.1001x; 1.4144x over previous
"""Trainium2 Bass kernel for nn_Attention_86268713108190.

7 independent attention "bands" over batch 8, n=512, d=512, 8 heads,
shared Wqkv/Wout. Sharding: data-parallel over batch -- core c handles
batch index c (7 band-samples of [512, 512] each).

Design (HW-measured 282us/core vs the 551us f32r baseline, rel err 4.6e-3
vs the 2e-2 gate; all decomposition numbers below are For_i
repeat-differenced on the axon-tunneled TRN2 cores):

* All matmuls in bf16: PE streams 1 col/cycle @2.4GHz vs ~2 for f32r.
  Inputs cast host-side (ml_dtypes); PSUM accumulation stays f32.
* Softmax denominator via 64 REPLICATED ones-columns in the AV lhsT
  (v_aug = [V_h | ones64] per head): the AV matmul output [128, 512] has
  rows 0:64 = O_h^T and rows 64:128 = den_h already broadcast across 64
  partitions, aligned with the O^T rows it must normalize. Zero extra PE
  streaming time (matmul cost is N-driven), no partition broadcast, no
  DRAM bounce, no gpsimd (vs baseline's 187us tail).
* 1/den = exp(-ln den) on ACT. NOT nc.vector.reciprocal: DVE's iterative
  reciprocal measured ~3.5 cycles/element (142us/core for all pairs) and
  blocks DVE's in-order FIFO. NOT AluOpType.divide: no DVE divide on
  TRN2 (walrus NCC_IXCG864). NOT ACT Reciprocal: blocked in bass and
  can't share a table set with Exp (777+1016 > 1536 bucket budget).
* _pin_act_table_set: insert_act_table_loads otherwise assigns Exp and
  Ln to DIFFERENT table sets and thrashes 43 ACT_TABLE_LOADs (~2.7us
  each, ~116us/iteration) -- masking the functions out of all sets
  except natural_log_exp_and_others yields exactly one load.
* Emission is software-pipelined per attention jt-slot of band s:
  [AV-pair(g-1), one filler chunk, S-pair(g)] where fillers are QKV
  projection groups of band s+1 and out-projection groups of band s-1.
  The exp-gated S sits LAST so independent PE work runs while ACT
  catches up. Both slot exps fuse into ONE N=1024 ACT op over a 2-bank
  [128, 2, 512] PSUM tile (saves ACT's 352-cycle per-op overhead).
* PSUM: psproj 1 (proj accumulators; fillers are spaced, so the single
  bank never stalls) + pss 2x2-bank (S, so S(jt+1) issues while exp(jt)
  runs) + pso 3x1-bank (AV pairs) = 8 banks exactly.

Per-band engine budgets (bf16): PE ~24us (112 matmuls; S pairs run
concurrently in separate row-groups via tile_position), ACT ~29us (16
merged exps + 12 recip ops), DVE ~17us (PSUM evictions, normalize muls,
bias adds). Measured ~40us/band steady state; residual is the
S->exp->S semaphore cadence in the attention chain.
"""

import contextlib
import sys

if '/opt/trn_rl_repo' not in sys.path:
    sys.path.insert(0, '/opt/trn_rl_repo')

import numpy as np


@contextlib.contextmanager
def _pin_act_table_set(names=("Exp", "Ln"), keep="natural_log_exp_and_others"):
    """Keep Exp+Ln servable only from the one table set that holds both, so
    insert_act_table_loads emits a single load instead of thrashing between
    exp_and_others and natural_log (~2.7us per switch, 43 switches/iter).
    Set order (= act_func_set_id indices) is preserved."""
    import concourse.bacc as bacc_mod
    import concourse.mybir as mybir
    fns = {getattr(mybir.ActivationFunctionType, n) for n in names}
    orig = bacc_mod.get_activation_tables

    def patched(arch):
        tables = dict(orig(arch))
        return {
            name: (fset if name == keep else fset - fns)
            for name, fset in tables.items()
        }

    bacc_mod.get_activation_tables = patched
    try:
        yield
    finally:
        bacc_mod.get_activation_tables = orig

P = 128
MM_DTYPE = "bf16"
NSEQ = 512
D = 512
H = 8
DH = 64
NBANDS = 7
NCORES = 8
SCALE = D ** -0.5


def build_kernel(nbands=NBANDS, repeat=1, mm_dtype=MM_DTYPE, interleave=True,
                 ablate="", tail="act"):
    import concourse.mybir as mybir
    import concourse.tile as tile
    from concourse import bacc

    f32 = mybir.dt.float32
    if mm_dtype == "bf16":
        mdt = mybir.dt.bfloat16
    elif mm_dtype == "f32r":
        mdt = mybir.dt.float32r
    else:
        mdt = mybir.dt.float32
    Exp = mybir.ActivationFunctionType.Exp
    Ln = mybir.ActivationFunctionType.Ln

    nc = bacc.Bacc("TRN2", target_bir_lowering=False, debug=False,
                   num_devices=NCORES)

    xT = nc.dram_tensor("xT", [nbands, D, NSEQ], mdt, kind="ExternalInput").ap()
    wqkvT = nc.dram_tensor("wqkvT", [D, 3 * D], mdt, kind="ExternalInput").ap()
    woutT = nc.dram_tensor("woutT", [D, D], mdt, kind="ExternalInput").ap()
    biasb = nc.dram_tensor("biasb", [P, D], f32, kind="ExternalInput").ap()
    out = nc.dram_tensor("out", [nbands, NSEQ, D], f32, kind="ExternalOutput").ap()

    with tile.TileContext(nc) as tc:
        with (
            tc.tile_pool(name="weights", bufs=1) as wpool,
            tc.tile_pool(name="x", bufs=3) as xpool,
            tc.tile_pool(name="qk", bufs=2) as qkpool,
            tc.tile_pool(name="ot", bufs=2) as otpool,
            tc.tile_pool(name="es", bufs=8) as spool,
            tc.tile_pool(name="ob", bufs=3) as outpool,
            tc.tile_pool(name="psproj", bufs=1, space="PSUM") as psproj,
            tc.tile_pool(name="pss", bufs=2, space="PSUM") as pss,
            tc.tile_pool(name="pso", bufs=3, space="PSUM") as pso,
            tc.tile_pool(name="rec", bufs=2) as recpool,
        ):
            wq_sb = wpool.tile([P, 4, 3 * D], mdt, name="wq_sb")
            wo_sb = wpool.tile([P, 4, D], mdt, name="wo_sb")
            bias_sb = wpool.tile([P, D], f32, name="bias_sb")
            # v_aug: per head 64 V columns + 64 ones columns, so the AV
            # matmul lands O^T on partitions 0:64 and the softmax
            # denominator REPLICATED on partitions 64:128. Two persistent
            # buffers alternated by band parity; ones half memset once.
            vas = [wpool.tile([P, 4, H, 2 * DH], mdt, name=f"va{i}")
                   for i in range(2)]
            for va in vas:
                ones = va[:, :, :, DH:2 * DH]
                if mm_dtype == "f32r":
                    ones = ones.bitcast(f32)
                nc.vector.memset(ones, 1.0)

            wq_r = wqkvT.rearrange("(ko ki) e -> ki ko e", ki=P)
            for kt in range(4):
                nc.sync.dma_start(wq_sb[:, kt, :], wq_r[:, kt, :])
            nc.sync.dma_start(wo_sb[:], woutT.rearrange("(ko ki) e -> ki ko e", ki=P))
            nc.sync.dma_start(bias_sb[:], biasb[:])

            def load_x(s):
                xt = xpool.tile([P, 4, NSEQ], mdt, tag="xt", name="xt")
                nc.sync.dma_start(
                    xt[:], xT[s].rearrange("(ko ki) n -> ki ko n", ki=P))
                return xt

            def qk_group(xt, qk_sb, et):
                ps = psproj.tile([P, NSEQ], f32, tag="psproj", name="ps")
                for kt in range(4):
                    nc.tensor.matmul(
                        ps[:], wq_sb[:, kt, et * P:(et + 1) * P], xt[:, kt, :],
                        start=(kt == 0), stop=(kt == 3))
                nc.vector.tensor_copy(qk_sb[:, et, :], ps[:])

            def v_group(xt, va, nt):
                ps = psproj.tile([P, NSEQ], f32, tag="psproj", name="ps")
                for kt in range(4):
                    nc.tensor.matmul(
                        ps[:], xt[:, kt, nt * P:(nt + 1) * P],
                        wq_sb[:, kt, 2 * D:3 * D],
                        start=(kt == 0), stop=(kt == 3))
                nc.vector.tensor_copy(
                    va[:, nt, :, 0:DH],
                    ps[:].rearrange("p (h dh) -> p h dh", h=H))

            def out_group(s, ot_sb, nt):
                ps = psproj.tile([P, NSEQ], f32, tag="psproj", name="ps")
                for kt in range(4):
                    nc.tensor.matmul(
                        ps[:], ot_sb[:, kt, nt * P:(nt + 1) * P],
                        wo_sb[:, kt, :],
                        start=(kt == 0), stop=(kt == 3))
                ob = outpool.tile([P, D], f32, tag="ob", name="ob")
                nc.vector.tensor_add(ob[:], ps[:], bias_sb[:])
                nc.sync.dma_start(
                    out[s].rearrange("(no ni) e -> ni no e", ni=P)[:, nt, :],
                    ob[:])

            def qk_chunks(s, xt):
                # q0,k0 first so the next band's pair-0 S can start early.
                for et in (0, 4, 1, 5, 2, 6, 3, 7):
                    yield lambda et=et: qk_group(xt, qk_sb_of[s], et)

            def v_chunks(xt, va):
                for nt in range(4):
                    yield lambda nt=nt: v_group(xt, va, nt)

            qk_sb_of = {}

            def emit_attention(s, qk_sb, va, filler):
                """S/exp for pair g interleaved with AV for pair g-1 and one
                filler chunk per jt slot; divides (the whole softmax
                normalize) inline on DVE."""
                es_store = {}
                ps_store = {}
                mul_queue = []

                def flush_muls():
                    while mul_queue:
                        g, ps_o, rec = mul_queue.pop(0)
                        nc.vector.tensor_mul(ot_sb[0:DH, g, :],
                                             ps_o[0:DH, 0, :], rec[:, 0, :])
                        nc.vector.tensor_mul(ot_sb[DH:P, g, :],
                                             ps_o[0:DH, 1, :], rec[:, 1, :])

                def s_pair(g, jt):
                    # one 2-bank PSUM tile for both halves -> ONE merged
                    # N=1024 exp (saves the 352-cycle ACT overhead per op)
                    ps_s = pss.tile([P, 2, NSEQ], f32, tag="pss", name="ps_s")
                    nc.tensor.matmul(
                        ps_s[:, 0, :], qk_sb[0:DH, 4 + g, jt * P:(jt + 1) * P],
                        qk_sb[0:DH, g, :], start=True, stop=True)
                    nc.tensor.matmul(
                        ps_s[:, 1, :], qk_sb[DH:P, 4 + g, jt * P:(jt + 1) * P],
                        qk_sb[DH:P, g, :], start=True, stop=True,
                        tile_position=(DH, 0))
                    es = spool.tile([P, 2, NSEQ], mdt, tag="es", name="es")
                    nc.scalar.activation(es[:], ps_s[:], Exp, scale=SCALE)
                    es_store.setdefault(g, []).append(es)

                def av_pair(g, jt):
                    if jt == 0:
                        ps_store[g] = (
                            pso.tile([P, NSEQ], f32, tag="pso", name="ps_o0"),
                            pso.tile([P, NSEQ], f32, tag="pso", name="ps_o1"))
                    ps_o0, ps_o1 = ps_store[g]
                    es = es_store[g][jt]
                    nc.tensor.matmul(
                        ps_o0[:], va[:, jt, 2 * g, :], es[:, 0, :],
                        start=(jt == 0), stop=(jt == 3))
                    nc.tensor.matmul(
                        ps_o1[:], va[:, jt, 2 * g + 1, :], es[:, 1, :],
                        start=(jt == 0), stop=(jt == 3))

                def divides(g):
                    # softmax normalize: rows 64:128 of ps_o hold the
                    # denominator replicated across 64 partitions, so this
                    # is a plain elementwise recip+mul -- no broadcasts.
                    # 1/den = exp(-ln den) on ACT (Ln+Exp share one table
                    # set -- pinned below so the load pass can't thrash);
                    # DVE's iterative reciprocal measured ~3.5 cyc/elem.
                    del es_store[g]
                    ps_o0, ps_o1 = ps_store.pop(g)
                    if ablate == "no_div":
                        nc.vector.tensor_copy(ot_sb[0:DH, g, :], ps_o0[0:DH, :])
                        nc.vector.tensor_copy(ot_sb[DH:P, g, :], ps_o1[0:DH, :])
                        return
                    rec = recpool.tile([DH, 2, NSEQ], f32, tag="rec",
                                       name="rec")
                    lg = recpool.tile([DH, 2, NSEQ], f32, tag="lg", name="lg")
                    nc.scalar.activation(lg[:, 0, :], ps_o0[DH:P, :], Ln)
                    nc.scalar.activation(lg[:, 1, :], ps_o1[DH:P, :], Ln)
                    nc.scalar.activation(rec[:], lg[:], Exp, scale=-1.0)
                    nc.vector.tensor_mul(ot_sb[0:DH, g, :], ps_o0[0:DH, :],
                                         rec[:, 0, :])
                    nc.vector.tensor_mul(ot_sb[DH:P, g, :], ps_o1[0:DH, :],
                                         rec[:, 1, :])

                ot_sb = otpool.tile([P, 4, NSEQ], mdt, tag="ot", name="ot_sb")
                if ablate == "no_attn":
                    nc.vector.tensor_copy(ot_sb[:], qk_sb[:, 0:4, :])
                    for f in filler:
                        f()
                    return ot_sb

                for g in range(5):
                    for jt in range(4):
                        if g >= 1:
                            av_pair(g - 1, jt)
                        f = next(filler, None)
                        if f is not None:
                            f()
                        if g < 4:
                            s_pair(g, jt)
                    if g >= 1:
                        divides(g - 1)
                flush_muls()
                for f in filler:
                    f()
                return ot_sb

            rep_ctx = (tc.For_i(0, repeat, 1,
                                hint_engines=(mybir.EngineType.PE,
                                              mybir.EngineType.Activation,
                                              mybir.EngineType.DVE))
                       if repeat > 1 else contextlib.nullcontext())
            with rep_ctx:
                xts = {0: load_x(0)}
                if nbands > 1:
                    xts[1] = load_x(1)
                # prologue: band 0's QKV emitted straight
                qk_sb_of[0] = qkpool.tile([P, 8, NSEQ], mdt, tag="qk",
                                          name="qk_sb")
                for f in qk_chunks(0, xts[0]):
                    f()
                for f in v_chunks(xts[0], vas[0]):
                    f()
                ot_prev = None
                for s in range(nbands):
                    if s + 2 < nbands:
                        xts[s + 2] = load_x(s + 2)
                    filler = []
                    if interleave:
                        if s + 1 < nbands:
                            qk_sb_of[s + 1] = qkpool.tile(
                                [P, 8, NSEQ], mdt, tag="qk", name="qk_sb")
                            filler.append(qk_chunks(s + 1, xts[s + 1]))
                        if ot_prev is not None:
                            op, os_ = ot_prev
                            filler.append(
                                lambda nt=nt, op=op, os_=os_: out_group(os_, op, nt)
                                for nt in range(4))
                        if s + 1 < nbands:
                            filler.append(v_chunks(xts[s + 1],
                                                   vas[(s + 1) % 2]))
                    fill_iter = (f for fl in filler for f in fl)
                    ot = emit_attention(s, qk_sb_of.pop(s), vas[s % 2],
                                        fill_iter)
                    if not interleave:
                        if s + 1 < nbands:
                            qk_sb_of[s + 1] = qkpool.tile(
                                [P, 8, NSEQ], mdt, tag="qk", name="qk_sb")
                            for f in qk_chunks(s + 1, xts[s + 1]):
                                f()
                            for f in v_chunks(xts[s + 1], vas[(s + 1) % 2]):
                                f()
                        if ot_prev is not None:
                            op, os_ = ot_prev
                            for nt in range(4):
                                out_group(os_, op, nt)
                    ot_prev = (ot, s)
                    xts.pop(s, None)
                # epilogue: last band's out-projection
                op, os_ = ot_prev
                for nt in range(4):
                    out_group(os_, op, nt)

    with _pin_act_table_set():
        nc.compile()
    return nc


_cached = None


def _get_nc():
    global _cached
    if _cached is None:
        _cached = build_kernel()
    return _cached


def make_in_maps(x, x_delta, x_theta, x_alpha, x_beta, x_gamma, x_upper,
                 Wqkv, Wout, bout, mm_dtype=MM_DTYPE):
    if mm_dtype == "bf16":
        import ml_dtypes
        cast_dt = ml_dtypes.bfloat16
    else:
        cast_dt = np.float32
    xs = np.stack([np.asarray(a, dtype=np.float32) for a in
                   (x, x_delta, x_theta, x_alpha, x_beta, x_gamma, x_upper)],
                  axis=0)  # [7, b, n, d]
    xsT = np.ascontiguousarray(xs.transpose(1, 0, 3, 2).astype(cast_dt))
    wqkvT = np.ascontiguousarray(np.asarray(Wqkv, np.float32).T.astype(cast_dt))
    woutT = np.ascontiguousarray(np.asarray(Wout, np.float32).T.astype(cast_dt))
    biasb = np.ascontiguousarray(
        np.broadcast_to(np.asarray(bout, np.float32)[None, :], (P, D)))
    return [
        {"xT": xsT[c], "wqkvT": wqkvT, "woutT": woutT, "biasb": biasb}
        for c in range(NCORES)
    ]


def kernel(x, x_delta, x_theta, x_alpha, x_beta, x_gamma, x_upper,
           Wqkv, Wout, bout):
    from concourse.bass_utils import run_bass_kernel_spmd

    nc = _get_nc()
    in_maps = make_in_maps(x, x_delta, x_theta, x_alpha, x_beta, x_gamma,
                           x_upper, Wqkv, Wout, bout)
    res = run_bass_kernel_spmd(nc, in_maps, core_ids=list(range(NCORES)))
    full = np.empty((NBANDS, NCORES, NSEQ, D), dtype=np.float32)
    for c in range(NCORES):
        full[:, c] = res.results[c]["out"]
    return tuple(full[i] for i in range(NBANDS))


# revision 14
# speedup vs baseline: 2.1564x; 1.0268x over previous
"""Trainium2 Bass kernel for nn_Attention_86268713108190.

7 independent attention "bands" over batch 8, n=512, d=512, 8 heads,
shared Wqkv/Wout. Sharding: data-parallel over batch -- core c handles
batch index c (7 band-samples of [512, 512] each).

Design (HW-measured 282us/core vs the 551us f32r baseline, rel err 4.6e-3
vs the 2e-2 gate; all decomposition numbers below are For_i
repeat-differenced on the axon-tunneled TRN2 cores):

* All matmuls in bf16: PE streams 1 col/cycle @2.4GHz vs ~2 for f32r.
  Inputs cast host-side (ml_dtypes); PSUM accumulation stays f32.
* Softmax denominator via 64 REPLICATED ones-columns in the AV lhsT
  (v_aug = [V_h | ones64] per head): the AV matmul output [128, 512] has
  rows 0:64 = O_h^T and rows 64:128 = den_h already broadcast across 64
  partitions, aligned with the O^T rows it must normalize. Zero extra PE
  streaming time (matmul cost is N-driven), no partition broadcast, no
  DRAM bounce, no gpsimd (vs baseline's 187us tail).
* 1/den = exp(-ln den) on ACT. NOT nc.vector.reciprocal: DVE's iterative
  reciprocal measured ~3.5 cycles/element (142us/core for all pairs) and
  blocks DVE's in-order FIFO. NOT AluOpType.divide: no DVE divide on
  TRN2 (walrus NCC_IXCG864). NOT ACT Reciprocal: blocked in bass and
  can't share a table set with Exp (777+1016 > 1536 bucket budget).
* _pin_act_table_set: insert_act_table_loads otherwise assigns Exp and
  Ln to DIFFERENT table sets and thrashes 43 ACT_TABLE_LOADs (~2.7us
  each, ~116us/iteration) -- masking the functions out of all sets
  except natural_log_exp_and_others yields exactly one load.
* Emission is software-pipelined per attention jt-slot of band s:
  [AV-pair(g-1), one filler chunk, S-pair(g)] where fillers are QKV
  projection groups of band s+1 and out-projection groups of band s-1.
  The exp-gated S sits LAST so independent PE work runs while ACT
  catches up. Both slot exps fuse into ONE N=1024 ACT op over a 2-bank
  [128, 2, 512] PSUM tile (saves ACT's 352-cycle per-op overhead).
* PSUM: psproj 1 (proj accumulators; fillers are spaced, so the single
  bank never stalls) + pss 2x2-bank (S, so S(jt+1) issues while exp(jt)
  runs) + pso 3x1-bank (AV pairs) = 8 banks exactly.

Per-band engine budgets (bf16): PE ~24us (112 matmuls; S pairs run
concurrently in separate row-groups via tile_position), ACT ~29us (16
merged exps + 12 recip ops), DVE ~17us (PSUM evictions, normalize muls,
bias adds). Measured ~40us/band steady state; residual is the
S->exp->S semaphore cadence in the attention chain.
"""

import contextlib
import sys

if '/opt/trn_rl_repo' not in sys.path:
    sys.path.insert(0, '/opt/trn_rl_repo')

import numpy as np


@contextlib.contextmanager
def _pin_act_table_set(names=("Exp", "Ln"), keep="natural_log_exp_and_others"):
    """Keep Exp+Ln servable only from the one table set that holds both, so
    insert_act_table_loads emits a single load instead of thrashing between
    exp_and_others and natural_log (~2.7us per switch, 43 switches/iter).
    Set order (= act_func_set_id indices) is preserved."""
    import concourse.bacc as bacc_mod
    import concourse.mybir as mybir
    fns = {getattr(mybir.ActivationFunctionType, n) for n in names}
    orig = bacc_mod.get_activation_tables

    def patched(arch):
        tables = dict(orig(arch))
        return {
            name: (fset if name == keep else fset - fns)
            for name, fset in tables.items()
        }

    bacc_mod.get_activation_tables = patched
    try:
        yield
    finally:
        bacc_mod.get_activation_tables = orig

P = 128
MM_DTYPE = "bf16"
NSEQ = 512
D = 512
H = 8
DH = 64
NBANDS = 7
NCORES = 8
SCALE = D ** -0.5


def build_kernel(nbands=NBANDS, repeat=1, mm_dtype=MM_DTYPE, interleave=True,
                 ablate="", tail="act"):
    import concourse.mybir as mybir
    import concourse.tile as tile
    from concourse import bacc

    f32 = mybir.dt.float32
    if mm_dtype == "bf16":
        mdt = mybir.dt.bfloat16
    elif mm_dtype == "f32r":
        mdt = mybir.dt.float32r
    else:
        mdt = mybir.dt.float32
    Exp = mybir.ActivationFunctionType.Exp
    Ln = mybir.ActivationFunctionType.Ln

    nc = bacc.Bacc("TRN2", target_bir_lowering=False, debug=False,
                   num_devices=NCORES)

    xT = nc.dram_tensor("xT", [nbands, D, NSEQ], mdt, kind="ExternalInput").ap()
    wqkvT = nc.dram_tensor("wqkvT", [D, 3 * D], mdt, kind="ExternalInput").ap()
    woutT = nc.dram_tensor("woutT", [D, D], mdt, kind="ExternalInput").ap()
    biasb = nc.dram_tensor("biasb", [P, D], f32, kind="ExternalInput").ap()
    out = nc.dram_tensor("out", [nbands, NSEQ, D], f32, kind="ExternalOutput").ap()

    with tile.TileContext(nc) as tc:
        with (
            tc.tile_pool(name="weights", bufs=1) as wpool,
            tc.tile_pool(name="x", bufs=3) as xpool,
            tc.tile_pool(name="qk", bufs=2) as qkpool,
            tc.tile_pool(name="ot", bufs=2) as otpool,
            tc.tile_pool(name="es", bufs=8) as spool,
            tc.tile_pool(name="ob", bufs=3) as outpool,
            tc.tile_pool(name="psproj", bufs=1, space="PSUM") as psproj,
            tc.tile_pool(name="pss", bufs=2, space="PSUM") as pss,
            tc.tile_pool(name="pso", bufs=3, space="PSUM") as pso,
            tc.tile_pool(name="rec", bufs=2) as recpool,
        ):
            wq_sb = wpool.tile([P, 4, 3 * D], mdt, name="wq_sb")
            wo_sb = wpool.tile([P, 4, D], mdt, name="wo_sb")
            bias_sb = wpool.tile([P, D], f32, name="bias_sb")
            # v_aug: per head 64 V columns + 64 ones columns, so the AV
            # matmul lands O^T on partitions 0:64 and the softmax
            # denominator REPLICATED on partitions 64:128. Two persistent
            # buffers alternated by band parity; ones half memset once.
            vas = [wpool.tile([P, 4, H, 2 * DH], mdt, name=f"va{i}")
                   for i in range(2)]
            for va in vas:
                ones = va[:, :, :, DH:2 * DH]
                if mm_dtype == "f32r":
                    ones = ones.bitcast(f32)
                nc.vector.memset(ones, 1.0)

            wq_r = wqkvT.rearrange("(ko ki) e -> ki ko e", ki=P)
            for kt in range(4):
                nc.sync.dma_start(wq_sb[:, kt, :], wq_r[:, kt, :])
            nc.sync.dma_start(wo_sb[:], woutT.rearrange("(ko ki) e -> ki ko e", ki=P))
            nc.sync.dma_start(bias_sb[:], biasb[:])

            def load_x(s):
                xt = xpool.tile([P, 4, NSEQ], mdt, tag="xt", name="xt")
                nc.sync.dma_start(
                    xt[:], xT[s].rearrange("(ko ki) n -> ki ko n", ki=P))
                return xt

            def qk_group(xt, qk_sb, et):
                ps = psproj.tile([P, NSEQ], f32, tag="psproj", name="ps")
                for kt in range(4):
                    nc.tensor.matmul(
                        ps[:], wq_sb[:, kt, et * P:(et + 1) * P], xt[:, kt, :],
                        start=(kt == 0), stop=(kt == 3))
                nc.vector.tensor_copy(qk_sb[:, et, :], ps[:])

            def v_group(xt, va, nt):
                ps = psproj.tile([P, NSEQ], f32, tag="psproj", name="ps")
                for kt in range(4):
                    nc.tensor.matmul(
                        ps[:], xt[:, kt, nt * P:(nt + 1) * P],
                        wq_sb[:, kt, 2 * D:3 * D],
                        start=(kt == 0), stop=(kt == 3))
                nc.vector.tensor_copy(
                    va[:, nt, :, 0:DH],
                    ps[:].rearrange("p (h dh) -> p h dh", h=H))

            def out_group(s, ot_sb, nt):
                ps = psproj.tile([P, NSEQ], f32, tag="psproj", name="ps")
                for kt in range(4):
                    nc.tensor.matmul(
                        ps[:], ot_sb[:, kt, nt * P:(nt + 1) * P],
                        wo_sb[:, kt, :],
                        start=(kt == 0), stop=(kt == 3))
                ob = outpool.tile([P, D], f32, tag="ob", name="ob")
                nc.vector.tensor_add(ob[:], ps[:], bias_sb[:])
                nc.sync.dma_start(
                    out[s].rearrange("(no ni) e -> ni no e", ni=P)[:, nt, :],
                    ob[:])

            def qk_chunks(s, xt):
                # q0,k0 first so the next band's pair-0 S can start early.
                for et in (0, 4, 1, 5, 2, 6, 3, 7):
                    yield lambda et=et: qk_group(xt, qk_sb_of[s], et)

            def v_chunks(xt, va):
                for nt in range(4):
                    yield lambda nt=nt: v_group(xt, va, nt)

            qk_sb_of = {}

            def emit_attention(s, qk_sb, va, filler):
                """S/exp for pair g interleaved with AV for pair g-1 and one
                filler chunk per jt slot; divides (the whole softmax
                normalize) inline on DVE."""
                es_store = {}
                ps_store = {}
                mul_queue = []

                def flush_muls():
                    while mul_queue:
                        g, ps_o, rec = mul_queue.pop(0)
                        nc.vector.tensor_mul(ot_sb[0:DH, g, :],
                                             ps_o[0:DH, 0, :], rec[:, 0, :])
                        nc.vector.tensor_mul(ot_sb[DH:P, g, :],
                                             ps_o[0:DH, 1, :], rec[:, 1, :])

                def s_pair(g, jt):
                    # one 2-bank PSUM tile for both halves -> ONE merged
                    # N=1024 exp (saves the 352-cycle ACT overhead per op)
                    ps_s = pss.tile([P, 2, NSEQ], f32, tag="pss", name="ps_s")
                    nc.tensor.matmul(
                        ps_s[:, 0, :], qk_sb[0:DH, 4 + g, jt * P:(jt + 1) * P],
                        qk_sb[0:DH, g, :], start=True, stop=True)
                    nc.tensor.matmul(
                        ps_s[:, 1, :], qk_sb[DH:P, 4 + g, jt * P:(jt + 1) * P],
                        qk_sb[DH:P, g, :], start=True, stop=True,
                        tile_position=(DH, 0))
                    es = spool.tile([P, 2, NSEQ], mdt, tag="es", name="es")
                    nc.scalar.activation(es[:], ps_s[:], Exp, scale=SCALE)
                    es_store.setdefault(g, []).append(es)

                def av_pair(g, jt):
                    if jt == 0:
                        ps_store[g] = (
                            pso.tile([P, NSEQ], f32, tag="pso", name="ps_o0"),
                            pso.tile([P, NSEQ], f32, tag="pso", name="ps_o1"))
                    ps_o0, ps_o1 = ps_store[g]
                    es = es_store[g][jt]
                    nc.tensor.matmul(
                        ps_o0[:], va[:, jt, 2 * g, :], es[:, 0, :],
                        start=(jt == 0), stop=(jt == 3))
                    nc.tensor.matmul(
                        ps_o1[:], va[:, jt, 2 * g + 1, :], es[:, 1, :],
                        start=(jt == 0), stop=(jt == 3))

                def divides(g):
                    # softmax normalize: rows 64:128 of ps_o hold the
                    # denominator replicated across 64 partitions, so this
                    # is a plain elementwise recip+mul -- no broadcasts.
                    # 1/den = exp(-ln den) on ACT (Ln+Exp share one table
                    # set -- pinned below so the load pass can't thrash);
                    # DVE's iterative reciprocal measured ~3.5 cyc/elem.
                    del es_store[g]
                    ps_o0, ps_o1 = ps_store.pop(g)
                    if ablate == "no_div":
                        nc.vector.tensor_copy(ot_sb[0:DH, g, :], ps_o0[0:DH, :])
                        nc.vector.tensor_copy(ot_sb[DH:P, g, :], ps_o1[0:DH, :])
                        return
                    rec = recpool.tile([DH, 2, NSEQ], f32, tag="rec",
                                       name="rec")
                    lg = recpool.tile([DH, 2, NSEQ], f32, tag="lg", name="lg")
                    nc.scalar.activation(lg[:, 0, :], ps_o0[DH:P, :], Ln)
                    nc.scalar.activation(lg[:, 1, :], ps_o1[DH:P, :], Ln)
                    nc.scalar.activation(rec[:], lg[:], Exp, scale=-1.0)
                    nc.vector.tensor_mul(ot_sb[0:DH, g, :], ps_o0[0:DH, :],
                                         rec[:, 0, :])
                    nc.vector.tensor_mul(ot_sb[DH:P, g, :], ps_o1[0:DH, :],
                                         rec[:, 1, :])

                ot_sb = otpool.tile([P, 4, NSEQ], mdt, tag="ot", name="ot_sb")
                if ablate == "no_attn":
                    nc.vector.tensor_copy(ot_sb[:], qk_sb[:, 0:4, :])
                    for f in filler:
                        f()
                    return ot_sb

                for g in range(5):
                    for jt in range(4):
                        if g >= 1:
                            av_pair(g - 1, jt)
                        f = next(filler, None)
                        if f is not None:
                            f()
                        if g < 4:
                            s_pair(g, jt)
                    if g >= 1:
                        divides(g - 1)
                flush_muls()
                for f in filler:
                    f()
                return ot_sb

            rep_ctx = (tc.For_i(0, repeat, 1,
                                hint_engines=(mybir.EngineType.PE,
                                              mybir.EngineType.Activation,
                                              mybir.EngineType.DVE))
                       if repeat > 1 else contextlib.nullcontext())
            with rep_ctx:
                xts = {0: load_x(0)}
                if nbands > 1:
                    xts[1] = load_x(1)
                # prologue: band 0's QKV emitted straight
                qk_sb_of[0] = qkpool.tile([P, 8, NSEQ], mdt, tag="qk",
                                          name="qk_sb")
                for f in qk_chunks(0, xts[0]):
                    f()
                for f in v_chunks(xts[0], vas[0]):
                    f()
                ot_prev = None
                for s in range(nbands):
                    if s + 2 < nbands:
                        xts[s + 2] = load_x(s + 2)
                    filler = []
                    if interleave:
                        if s + 1 < nbands:
                            qk_sb_of[s + 1] = qkpool.tile(
                                [P, 8, NSEQ], mdt, tag="qk", name="qk_sb")
                            filler.append(qk_chunks(s + 1, xts[s + 1]))
                        if ot_prev is not None:
                            op, os_ = ot_prev
                            filler.append(
                                lambda nt=nt, op=op, os_=os_: out_group(os_, op, nt)
                                for nt in range(4))
                        if s + 1 < nbands:
                            filler.append(v_chunks(xts[s + 1],
                                                   vas[(s + 1) % 2]))
                    fill_iter = (f for fl in filler for f in fl)
                    ot = emit_attention(s, qk_sb_of.pop(s), vas[s % 2],
                                        fill_iter)
                    if not interleave:
                        if s + 1 < nbands:
                            qk_sb_of[s + 1] = qkpool.tile(
                                [P, 8, NSEQ], mdt, tag="qk", name="qk_sb")
                            for f in qk_chunks(s + 1, xts[s + 1]):
                                f()
                            for f in v_chunks(xts[s + 1], vas[(s + 1) % 2]):
                                f()
                        if ot_prev is not None:
                            op, os_ = ot_prev
                            for nt in range(4):
                                out_group(os_, op, nt)
                    ot_prev = (ot, s)
                    xts.pop(s, None)
                # epilogue: last band's out-projection
                op, os_ = ot_prev
                for nt in range(4):
                    out_group(os_, op, nt)

    with _pin_act_table_set():
        nc.compile()
    return nc


_cached = None


def _get_nc():
    global _cached
    if _cached is None:
        _cached = build_kernel()
    return _cached


def make_in_maps(x, x_delta, x_theta, x_alpha, x_beta, x_gamma, x_upper,
                 Wqkv, Wout, bout, mm_dtype=MM_DTYPE):
    if mm_dtype == "bf16":
        import ml_dtypes
        cast_dt = ml_dtypes.bfloat16
    else:
        cast_dt = np.float32
    xs = np.stack([np.asarray(a, dtype=np.float32) for a in
                   (x, x_delta, x_theta, x_alpha, x_beta, x_gamma, x_upper)],
                  axis=0)  # [7, b, n, d]
    xsT = np.ascontiguousarray(xs.transpose(1, 0, 3, 2).astype(cast_dt))
    wqkvT = np.ascontiguousarray(np.asarray(Wqkv, np.float32).T.astype(cast_dt))
    woutT = np.ascontiguousarray(np.asarray(Wout, np.float32).T.astype(cast_dt))
    biasb = np.ascontiguousarray(
        np.broadcast_to(np.asarray(bout, np.float32)[None, :], (P, D)))
    return [
        {"xT": xsT[c], "wqkvT": wqkvT, "woutT": woutT, "biasb": biasb}
        for c in range(NCORES)
    ]


def kernel(x, x_delta, x_theta, x_alpha, x_beta, x_gamma, x_upper,
           Wqkv, Wout, bout):
    from concourse.bass_utils import run_bass_kernel_spmd

    nc = _get_nc()
    in_maps = make_in_maps(x, x_delta, x_theta, x_alpha, x_beta, x_gamma,
                           x_upper, Wqkv, Wout, bout)
    res = run_bass_kernel_spmd(nc, in_maps, core_ids=list(range(NCORES)))
    full = np.empty((NBANDS, NCORES, NSEQ, D), dtype=np.float32)
    for c in range(NCORES):
        full[:, c] = res.results[c]["out"]
    return tuple(full[i] for i in range(NBANDS))


# revision 15
# speedup vs baseline: 2.2420x; 1.0397x over previous
"""Trainium2 Bass kernel for nn_Attention_86268713108190.

7 independent attention "bands" over batch 8, n=512, d=512, 8 heads,
shared Wqkv/Wout. Sharding: data-parallel over batch -- core c handles
batch index c (7 band-samples of [512, 512] each).

Design (HW-measured 282us/core vs the 551us f32r baseline, rel err 4.6e-3
vs the 2e-2 gate; all decomposition numbers below are For_i
repeat-differenced on the axon-tunneled TRN2 cores):

* All matmuls in bf16: PE streams 1 col/cycle @2.4GHz vs ~2 for f32r.
  Inputs cast host-side (ml_dtypes); PSUM accumulation stays f32.
* Softmax denominator via 64 REPLICATED ones-columns in the AV lhsT
  (v_aug = [V_h | ones64] per head): the AV matmul output [128, 512] has
  rows 0:64 = O_h^T and rows 64:128 = den_h already broadcast across 64
  partitions, aligned with the O^T rows it must normalize. Zero extra PE
  streaming time (matmul cost is N-driven), no partition broadcast, no
  DRAM bounce, no gpsimd (vs baseline's 187us tail).
* 1/den = exp(-ln den) on ACT. NOT nc.vector.reciprocal: DVE's iterative
  reciprocal measured ~3.5 cycles/element (142us/core for all pairs) and
  blocks DVE's in-order FIFO. NOT AluOpType.divide: no DVE divide on
  TRN2 (walrus NCC_IXCG864). NOT ACT Reciprocal: blocked in bass and
  can't share a table set with Exp (777+1016 > 1536 bucket budget).
* _pin_act_table_set: insert_act_table_loads otherwise assigns Exp and
  Ln to DIFFERENT table sets and thrashes 43 ACT_TABLE_LOADs (~2.7us
  each, ~116us/iteration) -- masking the functions out of all sets
  except natural_log_exp_and_others yields exactly one load.
* Emission is software-pipelined per attention jt-slot of band s:
  [AV-pair(g-1), one filler chunk, S-pair(g)] where fillers are QKV
  projection groups of band s+1 and out-projection groups of band s-1.
  The exp-gated S sits LAST so independent PE work runs while ACT
  catches up. Both slot exps fuse into ONE N=1024 ACT op over a 2-bank
  [128, 2, 512] PSUM tile (saves ACT's 352-cycle per-op overhead).
* PSUM: psproj 1 (proj accumulators; fillers are spaced, so the single
  bank never stalls) + pss 2x2-bank (S, so S(jt+1) issues while exp(jt)
  runs) + pso 3x1-bank (AV pairs) = 8 banks exactly.

Per-band engine budgets (bf16): PE ~24us (112 matmuls; S pairs run
concurrently in separate row-groups via tile_position), ACT ~29us (16
merged exps + 12 recip ops), DVE ~17us (PSUM evictions, normalize muls,
bias adds). Measured ~40us/band steady state; residual is the
S->exp->S semaphore cadence in the attention chain.
"""

import contextlib
import sys

if '/opt/trn_rl_repo' not in sys.path:
    sys.path.insert(0, '/opt/trn_rl_repo')

import numpy as np


@contextlib.contextmanager
def _pin_act_table_set(names=("Exp", "Ln"), keep="natural_log_exp_and_others"):
    """Keep Exp+Ln servable only from the one table set that holds both, so
    insert_act_table_loads emits a single load instead of thrashing between
    exp_and_others and natural_log (~2.7us per switch, 43 switches/iter).
    Set order (= act_func_set_id indices) is preserved."""
    import concourse.bacc as bacc_mod
    import concourse.mybir as mybir
    fns = {getattr(mybir.ActivationFunctionType, n) for n in names}
    orig = bacc_mod.get_activation_tables

    def patched(arch):
        tables = dict(orig(arch))
        return {
            name: (fset if name == keep else fset - fns)
            for name, fset in tables.items()
        }

    bacc_mod.get_activation_tables = patched
    try:
        yield
    finally:
        bacc_mod.get_activation_tables = orig

P = 128
MM_DTYPE = "bf16"
NSEQ = 512
D = 512
H = 8
DH = 64
NBANDS = 7
NCORES = 8
SCALE = D ** -0.5


def build_kernel(nbands=NBANDS, repeat=1, mm_dtype=MM_DTYPE, interleave=True,
                 ablate="", tail="act"):
    import concourse.mybir as mybir
    import concourse.tile as tile
    from concourse import bacc

    f32 = mybir.dt.float32
    if mm_dtype == "bf16":
        mdt = mybir.dt.bfloat16
    elif mm_dtype == "f32r":
        mdt = mybir.dt.float32r
    else:
        mdt = mybir.dt.float32
    Exp = mybir.ActivationFunctionType.Exp
    Ln = mybir.ActivationFunctionType.Ln

    nc = bacc.Bacc("TRN2", target_bir_lowering=False, debug=False,
                   num_devices=NCORES)

    xT = nc.dram_tensor("xT", [nbands, D, NSEQ], mdt, kind="ExternalInput").ap()
    wqkvT = nc.dram_tensor("wqkvT", [D, 3 * D], mdt, kind="ExternalInput").ap()
    woutT = nc.dram_tensor("woutT", [D, D], mdt, kind="ExternalInput").ap()
    biasb = nc.dram_tensor("biasb", [P, D], f32, kind="ExternalInput").ap()
    out = nc.dram_tensor("out", [nbands, NSEQ, D], f32, kind="ExternalOutput").ap()

    with tile.TileContext(nc) as tc:
        with (
            tc.tile_pool(name="weights", bufs=1) as wpool,
            tc.tile_pool(name="x", bufs=3) as xpool,
            tc.tile_pool(name="qk", bufs=2) as qkpool,
            tc.tile_pool(name="ot", bufs=2) as otpool,
            tc.tile_pool(name="es", bufs=8) as spool,
            tc.tile_pool(name="ob", bufs=3) as outpool,
            tc.tile_pool(name="psproj", bufs=1, space="PSUM") as psproj,
            tc.tile_pool(name="pss", bufs=2, space="PSUM") as pss,
            tc.tile_pool(name="pso", bufs=3, space="PSUM") as pso,
            tc.tile_pool(name="rec", bufs=2) as recpool,
        ):
            wq_sb = wpool.tile([P, 4, 3 * D], mdt, name="wq_sb")
            wo_sb = wpool.tile([P, 4, D], mdt, name="wo_sb")
            bias_sb = wpool.tile([P, D], f32, name="bias_sb")
            # v_aug: per head 64 V columns + 64 ones columns, so the AV
            # matmul lands O^T on partitions 0:64 and the softmax
            # denominator REPLICATED on partitions 64:128. Two persistent
            # buffers alternated by band parity; ones half memset once.
            vas = [wpool.tile([P, 4, H, 2 * DH], mdt, name=f"va{i}")
                   for i in range(2)]
            for va in vas:
                ones = va[:, :, :, DH:2 * DH]
                if mm_dtype == "f32r":
                    ones = ones.bitcast(f32)
                nc.vector.memset(ones, 1.0)

            wq_r = wqkvT.rearrange("(ko ki) e -> ki ko e", ki=P)
            for kt in range(4):
                nc.sync.dma_start(wq_sb[:, kt, :], wq_r[:, kt, :])
            nc.sync.dma_start(wo_sb[:], woutT.rearrange("(ko ki) e -> ki ko e", ki=P))
            nc.sync.dma_start(bias_sb[:], biasb[:])

            def load_x(s):
                xt = xpool.tile([P, 4, NSEQ], mdt, tag="xt", name="xt")
                nc.sync.dma_start(
                    xt[:], xT[s].rearrange("(ko ki) n -> ki ko n", ki=P))
                return xt

            def qk_group(xt, qk_sb, et, boundary=False):
                pool, tag = (pso, "pso") if boundary else (psproj, "psproj")
                ps = pool.tile([P, NSEQ], f32, tag=tag, name="ps")
                for kt in range(4):
                    nc.tensor.matmul(
                        ps[:], wq_sb[:, kt, et * P:(et + 1) * P], xt[:, kt, :],
                        start=(kt == 0), stop=(kt == 3))
                nc.vector.tensor_copy(qk_sb[:, et, :], ps[:])

            def v_group(xt, va, nt, boundary=False):
                pool, tag = (pso, "pso") if boundary else (psproj, "psproj")
                ps = pool.tile([P, NSEQ], f32, tag=tag, name="ps")
                for kt in range(4):
                    nc.tensor.matmul(
                        ps[:], xt[:, kt, nt * P:(nt + 1) * P],
                        wq_sb[:, kt, 2 * D:3 * D],
                        start=(kt == 0), stop=(kt == 3))
                nc.vector.tensor_copy(
                    va[:, nt, :, 0:DH],
                    ps[:].rearrange("p (h dh) -> p h dh", h=H))

            def out_group(s, ot_sb, nt):
                ps = psproj.tile([P, NSEQ], f32, tag="psproj", name="ps")
                for kt in range(4):
                    nc.tensor.matmul(
                        ps[:], ot_sb[:, kt, nt * P:(nt + 1) * P],
                        wo_sb[:, kt, :],
                        start=(kt == 0), stop=(kt == 3))
                ob = outpool.tile([P, D], f32, tag="ob", name="ob")
                nc.vector.tensor_add(ob[:], ps[:], bias_sb[:])
                nc.sync.dma_start(
                    out[s].rearrange("(no ni) e -> ni no e", ni=P)[:, nt, :],
                    ob[:])

            def qk_chunks(s, xt):
                # q0,k0 first so the next band's pair-0 S can start early.
                for et in (0, 4, 1, 5, 2, 6, 3, 7):
                    yield lambda b=False, et=et: qk_group(xt, qk_sb_of[s],
                                                          et, boundary=b)

            def v_chunks(xt, va):
                for nt in range(4):
                    yield lambda b=False, nt=nt: v_group(xt, va, nt,
                                                         boundary=b)

            qk_sb_of = {}

            def emit_attention(s, qk_sb, va, filler):
                """S/exp for pair g interleaved with AV for pair g-1 and one
                filler chunk per jt slot; divides (the whole softmax
                normalize) inline on DVE."""
                es_store = {}
                ps_store = {}
                mul_queue = []

                def flush_muls():
                    while mul_queue:
                        g, ps_o, rec = mul_queue.pop(0)
                        nc.vector.tensor_mul(ot_sb[0:DH, g, :],
                                             ps_o[0:DH, 0, :], rec[:, 0, :])
                        nc.vector.tensor_mul(ot_sb[DH:P, g, :],
                                             ps_o[0:DH, 1, :], rec[:, 1, :])

                def s_pair(g, jt):
                    # one 2-bank PSUM tile for both halves -> ONE merged
                    # N=1024 exp (saves the 352-cycle ACT overhead per op)
                    ps_s = pss.tile([P, 2, NSEQ], f32, tag="pss", name="ps_s")
                    nc.tensor.matmul(
                        ps_s[:, 0, :], qk_sb[0:DH, 4 + g, jt * P:(jt + 1) * P],
                        qk_sb[0:DH, g, :], start=True, stop=True)
                    nc.tensor.matmul(
                        ps_s[:, 1, :], qk_sb[DH:P, 4 + g, jt * P:(jt + 1) * P],
                        qk_sb[DH:P, g, :], start=True, stop=True,
                        tile_position=(DH, 0))
                    es = spool.tile([P, 2, NSEQ], mdt, tag="es", name="es")
                    nc.scalar.activation(es[:], ps_s[:], Exp, scale=SCALE)
                    es_store.setdefault(g, []).append(es)

                def av_pair(g, jt):
                    if jt == 0:
                        ps_store[g] = (
                            pso.tile([P, NSEQ], f32, tag="pso", name="ps_o0"),
                            pso.tile([P, NSEQ], f32, tag="pso", name="ps_o1"))
                    ps_o0, ps_o1 = ps_store[g]
                    es = es_store[g][jt]
                    nc.tensor.matmul(
                        ps_o0[:], va[:, jt, 2 * g, :], es[:, 0, :],
                        start=(jt == 0), stop=(jt == 3))
                    nc.tensor.matmul(
                        ps_o1[:], va[:, jt, 2 * g + 1, :], es[:, 1, :],
                        start=(jt == 0), stop=(jt == 3))

                def divides(g):
                    # softmax normalize: rows 64:128 of ps_o hold the
                    # denominator replicated across 64 partitions, so this
                    # is a plain elementwise recip+mul -- no broadcasts.
                    # 1/den = exp(-ln den) on ACT (Ln+Exp share one table
                    # set -- pinned below so the load pass can't thrash);
                    # DVE's iterative reciprocal measured ~3.5 cyc/elem.
                    del es_store[g]
                    ps_o0, ps_o1 = ps_store.pop(g)
                    if ablate == "no_div":
                        nc.vector.tensor_copy(ot_sb[0:DH, g, :], ps_o0[0:DH, :])
                        nc.vector.tensor_copy(ot_sb[DH:P, g, :], ps_o1[0:DH, :])
                        return
                    rec = recpool.tile([DH, 2, NSEQ], f32, tag="rec",
                                       name="rec")
                    lg = recpool.tile([DH, 2, NSEQ], f32, tag="lg", name="lg")
                    nc.scalar.activation(lg[:, 0, :], ps_o0[DH:P, :], Ln)
                    nc.scalar.activation(lg[:, 1, :], ps_o1[DH:P, :], Ln)
                    nc.scalar.activation(rec[:], lg[:], Exp, scale=-1.0)
                    nc.vector.tensor_mul(ot_sb[0:DH, g, :], ps_o0[0:DH, :],
                                         rec[:, 0, :])
                    nc.vector.tensor_mul(ot_sb[DH:P, g, :], ps_o1[0:DH, :],
                                         rec[:, 1, :])

                ot_sb = otpool.tile([P, 4, NSEQ], mdt, tag="ot", name="ot_sb")
                if ablate == "no_attn":
                    nc.vector.tensor_copy(ot_sb[:], qk_sb[:, 0:4, :])
                    for f in filler:
                        f()
                    return ot_sb

                for g in range(5):
                    for jt in range(4):
                        if g >= 1:
                            av_pair(g - 1, jt)
                        f = next(filler, None)
                        if f is not None:
                            f()
                        if g == 0:
                            f2 = next(filler, None)
                            if f2 is not None:
                                f2(True)
                        if g < 4:
                            s_pair(g, jt)
                    if g >= 1:
                        divides(g - 1)
                flush_muls()
                for f in filler:
                    f()
                return ot_sb

            rep_ctx = (tc.For_i(0, repeat, 1,
                                hint_engines=(mybir.EngineType.PE,
                                              mybir.EngineType.Activation,
                                              mybir.EngineType.DVE))
                       if repeat > 1 else contextlib.nullcontext())
            with rep_ctx:
                xts = {0: load_x(0)}
                if nbands > 1:
                    xts[1] = load_x(1)
                # prologue: band 0's QKV emitted straight
                qk_sb_of[0] = qkpool.tile([P, 8, NSEQ], mdt, tag="qk",
                                          name="qk_sb")
                for f in qk_chunks(0, xts[0]):
                    f()
                for f in v_chunks(xts[0], vas[0]):
                    f()
                ot_prev = None
                for s in range(nbands):
                    if s + 2 < nbands:
                        xts[s + 2] = load_x(s + 2)
                    filler = []
                    if interleave:
                        if s + 1 < nbands:
                            qk_sb_of[s + 1] = qkpool.tile(
                                [P, 8, NSEQ], mdt, tag="qk", name="qk_sb")
                            filler.append(qk_chunks(s + 1, xts[s + 1]))
                        if ot_prev is not None:
                            op, os_ = ot_prev
                            filler.append(
                                lambda b=False, nt=nt, op=op, os_=os_:
                                    out_group(os_, op, nt)
                                for nt in range(4))
                        if s + 1 < nbands:
                            filler.append(v_chunks(xts[s + 1],
                                                   vas[(s + 1) % 2]))
                    fill_iter = (f for fl in filler for f in fl)
                    ot = emit_attention(s, qk_sb_of.pop(s), vas[s % 2],
                                        fill_iter)
                    if not interleave:
                        if s + 1 < nbands:
                            qk_sb_of[s + 1] = qkpool.tile(
                                [P, 8, NSEQ], mdt, tag="qk", name="qk_sb")
                            for f in qk_chunks(s + 1, xts[s + 1]):
                                f()
                            for f in v_chunks(xts[s + 1], vas[(s + 1) % 2]):
                                f()
                        if ot_prev is not None:
                            op, os_ = ot_prev
                            for nt in range(4):
                                out_group(os_, op, nt)
                    ot_prev = (ot, s)
                    xts.pop(s, None)
                # epilogue: last band's out-projection
                op, os_ = ot_prev
                for nt in range(4):
                    out_group(os_, op, nt)

    with _pin_act_table_set():
        nc.compile()
    return nc


_cached = None


def _get_nc():
    global _cached
    if _cached is None:
        _cached = build_kernel()
    return _cached


def make_in_maps(x, x_delta, x_theta, x_alpha, x_beta, x_gamma, x_upper,
                 Wqkv, Wout, bout, mm_dtype=MM_DTYPE):
    if mm_dtype == "bf16":
        import ml_dtypes
        cast_dt = ml_dtypes.bfloat16
    else:
        cast_dt = np.float32
    xs = np.stack([np.asarray(a, dtype=np.float32) for a in
                   (x, x_delta, x_theta, x_alpha, x_beta, x_gamma, x_upper)],
                  axis=0)  # [7, b, n, d]
    xsT = np.ascontiguousarray(xs.transpose(1, 0, 3, 2).astype(cast_dt))
    wqkvT = np.ascontiguousarray(np.asarray(Wqkv, np.float32).T.astype(cast_dt))
    woutT = np.ascontiguousarray(np.asarray(Wout, np.float32).T.astype(cast_dt))
    biasb = np.ascontiguousarray(
        np.broadcast_to(np.asarray(bout, np.float32)[None, :], (P, D)))
    return [
        {"xT": xsT[c], "wqkvT": wqkvT, "woutT": woutT, "biasb": biasb}
        for c in range(NCORES)
    ]


def kernel(x, x_delta, x_theta, x_alpha, x_beta, x_gamma, x_upper,
           Wqkv, Wout, bout):
    from concourse.bass_utils import run_bass_kernel_spmd

    nc = _get_nc()
    in_maps = make_in_maps(x, x_delta, x_theta, x_alpha, x_beta, x_gamma,
                           x_upper, Wqkv, Wout, bout)
    res = run_bass_kernel_spmd(nc, in_maps, core_ids=list(range(NCORES)))
    full = np.empty((NBANDS, NCORES, NSEQ, D), dtype=np.float32)
    for c in range(NCORES):
        full[:, c] = res.results[c]["out"]
    return tuple(full[i] for i in range(NBANDS))
